# revision 22
# baseline (speedup 1.0000x reference)
"""FP8 block-wise dequant linear: out[b,s,o] = sum_i x[b,s,i] * (w[o,i]*scale[o//128,i//128]).

Sharding: 4-way over seq x 2-way over out_features across 8 NeuronCores.
Per core: x shard [512 seq, 4096 in] (bf16, host-precast), w shard
[2048 out, 4096 in] fp8, out [512, 2048] f32. All DRAM buffers are
host-staged partition-major (contraction dim on partitions, 128 KiB
contiguous per partition) so SWDGE DMAs spray across all 16 SDMA engines.

Device pipeline per core:
  - GpSimd (SWDGE) DMAs x bf16 straight into a resident SBUF tile.
  - GpSimd DMAs w fp8 slabs; VectorE dequantizes to bf16 via tensor_tensor
    with a free-dim-broadcast per-128x128-block scale operand (resident).
  - TensorE: per seq block, kb outer / out-chunk inner, so each stationary
    x-block load feeds 4 N=512 bf16 matmuls; 32 kb accumulate in 4 PSUM
    banks; ScalarE evacuates; GpSimd DMAs out.

Measured (8-core SPMD, per-core): ~131 us vs ~109 us bf16 PE roofline.
"""

import numpy as np
import ml_dtypes

import concourse.bacc as bacc
import concourse.mybir as mybir
from concourse.tile import TileContext
from concourse.bass_utils import run_bass_kernel_spmd

SEQ, DIN, DOUT = 2048, 4096, 4096
N_CORES = 8
SEQ_SHARDS, OUT_SHARDS = 4, 2
SEQ_SH, OUT_SH = SEQ // SEQ_SHARDS, DOUT // OUT_SHARDS  # 1024, 1024
P = 128
NKB = DIN // P            # 32 contraction blocks
NOB = OUT_SH // P         # 8 out blocks per core
NMM = 512                 # matmul moving free dim
NOC = OUT_SH // NMM       # 2 out chunks per core
NSB = SEQ_SH // P         # 8 seq blocks per core


XPIECES = 4   # x DMA transfers (bf16, host-precast), 2 MiB each
WCHUNK = 4    # kb slabs per w DMA (1 MiB fp8 transfers)
DMA_ENGINE = "gpsimd"


def _dma(nc):
    return getattr(nc, DMA_ENGINE)


def emit_load(nc, pools, io, nkb=NKB):
    """x arrives bf16 (host-precast, same RNE rounding the on-chip cast would
    do) and is DMA'd straight into the resident activation tile. w arrives
    fp8, staged, and dequantized to bf16 by VectorE tensor_tensor with a
    free-dim-broadcast per-block scale. All DRAM is host-staged
    partition-major so transfers spray across all 16 SDMA engines."""
    dt = mybir.dt
    persist, wf_pool, ob_pool, ps_pool = pools
    xt, wt, sc, out = io

    sc_sb = persist.tile([P, NKB * NOB], dt.float32, tag="sc")
    nc.sync.dma_start(sc_sb[:], sc[:])

    xb_all = persist.tile([P, NKB * SEQ_SH], dt.bfloat16, tag="xb")
    wq_all = persist.tile([P, NKB * OUT_SH], dt.bfloat16, tag="wq")

    wq = []
    xb = []
    assert nkb % XPIECES == 0
    xstep = nkb // XPIECES
    for i in range(XPIECES):
        lo, hi = i * xstep * SEQ_SH, (i + 1) * xstep * SEQ_SH
        _dma(nc).dma_start(xb_all[:, lo:hi], xt[:, lo:hi])

    for kb0 in range(0, nkb, WCHUNK):
        nb = min(WCHUNK, nkb - kb0)
        wf = wf_pool.tile([P, WCHUNK * OUT_SH], dt.float8e4, tag="wf")
        _dma(nc).dma_start(
            wf[:, :nb * OUT_SH],
            wt[:, kb0 * OUT_SH:(kb0 + nb) * OUT_SH])
        for j in range(nb):
            kb = kb0 + j
            s_b = (sc_sb[:, kb * NOB:(kb + 1) * NOB]
                   .unsqueeze(2).broadcast_to([P, NOB, P]))
            wslab = wq_all[:, kb * OUT_SH:(kb + 1) * OUT_SH]
            nc.vector.tensor_mul(
                wslab.rearrange("p (b i) -> p b i", b=NOB),
                wf[:, j * OUT_SH:(j + 1) * OUT_SH]
                .rearrange("p (b i) -> p b i", b=NOB),
                s_b,
            )
            wq.append(wslab)
    for kb in range(nkb):
        xb.append(xb_all[:, kb * SEQ_SH:(kb + 1) * SEQ_SH])
    return xb, wq


def emit_mm(nc, pools, io, xb, wq, nkb=NKB, nsb=NSB, noc=NOC):
    """Accumulating matmuls + PSUM evacuation + output DMA.

    Loop order: for each seq block, kb is the outer loop and the out-chunks
    are inner, so consecutive matmuls share the stationary operand xb[kb][sb]
    and its weight load amortizes/overlaps.

    out is host-staged partition-major: out[p, sb*OUT_SH+o] = y[sb*128+p, o]."""
    dt = mybir.dt
    persist, wf_pool, ob_pool, ps_pool = pools
    xt, wt, sc, out = io
    for sb in range(nsb):
        pss = []
        for oc in range(noc):
            ps = ps_pool.tile([P, NMM], dt.float32, tag="ps")
            pss.append(ps)
        for kb in range(nkb):
            lhs = xb[kb][:, sb * P:(sb + 1) * P]
            for oc in range(noc):
                nc.tensor.matmul(
                    pss[oc][:],
                    lhs,
                    wq[kb][:, oc * NMM:(oc + 1) * NMM],
                    start=(kb == 0),
                    stop=(kb == nkb - 1),
                )
        for half in range(0, noc, 2):
            ob = ob_pool.tile([P, 2 * NMM], dt.float32, tag="ob")
            for j in range(min(2, noc - half)):
                nc.scalar.copy(ob[:, j * NMM:(j + 1) * NMM], pss[half + j][:])
            _dma(nc).dma_start(
                out[:, sb * (NOC * NMM) + half * NMM:
                       sb * (NOC * NMM) + (half + min(2, noc - half)) * NMM],
                ob[:, :min(2, noc - half) * NMM])


def emit_body(nc, tc, pools, io, it, nkb=NKB, nsb=NSB, noc=NOC,
              do_mm=True, do_load=True):
    dt = mybir.dt
    persist, wf_pool, ob_pool, ps_pool = pools
    xt, wt, sc, out = io
    if do_load:
        xb, wq = emit_load(nc, pools, io, nkb=nkb)
    else:
        xb_all = persist.tile([P, NKB * SEQ_SH], dt.bfloat16, tag="xb")
        wq_all = persist.tile([P, NKB * OUT_SH], dt.bfloat16, tag="wq")
        xb = [xb_all[:, kb * SEQ_SH:(kb + 1) * SEQ_SH] for kb in range(nkb)]
        wq = [wq_all[:, kb * OUT_SH:(kb + 1) * OUT_SH] for kb in range(nkb)]
    if not do_mm:
        ob = ob_pool.tile([P, NMM], dt.float32, tag="ob")
        nc.vector.tensor_copy(ob[:], wq[0][:, 0:NMM])
        nc.sync.dma_start(out[0:P, 0:NMM], ob[:])
        return
    emit_mm(nc, pools, io, xb, wq, nkb=nkb, nsb=nsb, noc=noc)


def build_nc(iters=1, loop=None, **kw):
    nc = bacc.Bacc(None, target_bir_lowering=False)
    xt = nc.dram_tensor("xt", [P, NKB * SEQ_SH], mybir.dt.bfloat16, kind="ExternalInput")
    wt = nc.dram_tensor("wt", [P, NKB * OUT_SH], mybir.dt.float8e4, kind="ExternalInput")
    sc = nc.dram_tensor("sc", [P, NKB * NOB], mybir.dt.float32, kind="ExternalInput")
    out = nc.dram_tensor("out", [P, NSB * OUT_SH], mybir.dt.float32,
                         kind="ExternalOutput")
    io = (xt, wt, sc, out)

    with TileContext(nc) as tc:
        with (
            tc.tile_pool(name="persist", bufs=1) as persist,
            tc.tile_pool(name="wf", bufs=2) as wf_pool,
            tc.tile_pool(name="ob", bufs=3) as ob_pool,
            tc.tile_pool(name="ps", bufs=8, space="PSUM") as ps_pool,
        ):
            pools = (persist, wf_pool, ob_pool, ps_pool)
            if loop is not None:
                phase = kw.pop("loop_phase", "all")
                if phase == "mm":
                    xb, wq = emit_load(nc, pools, io)
                    with tc.For_i(0, loop, 1):
                        emit_mm(nc, pools, io, xb, wq)
                elif phase == "load":
                    with tc.For_i(0, loop, 1):
                        emit_body(nc, tc, pools, io, 0, do_mm=False, **kw)
                else:
                    with tc.For_i(0, loop, 1):
                        emit_body(nc, tc, pools, io, 0, **kw)
            else:
                for it in range(iters):
                    emit_body(nc, tc, pools, io, it, **kw)
    nc.compile()
    return nc


def shard_inputs(x, weight, weight_scale_inv):
    """Host staging, partition-major per core:
       xt[p, kb*SEQ_SH+f] = x[0][si*SEQ_SH+f, kb*128+p]
       wt[p, kb*OUT_SH+o] = weight[oi*OUT_SH+o, kb*128+p]
       sc[p, kb*NOB+ob]   = weight_scale_inv[oi*NOB+ob, kb]"""
    x = np.asarray(x)
    weight = np.asarray(weight)
    scale = np.asarray(weight_scale_inv, dtype=np.float32)
    w8 = weight.view(np.uint8)

    in_maps = []
    x_dev = {}
    w_dev = {}
    for c in range(N_CORES):
        si, oi = c % SEQ_SHARDS, c // SEQ_SHARDS
        if si not in x_dev:
            xs = np.asarray(x[0][si * SEQ_SH:(si + 1) * SEQ_SH, :],
                            dtype=np.float32).astype(ml_dtypes.bfloat16)
            x_dev[si] = np.ascontiguousarray(
                xs.T.reshape(NKB, P, SEQ_SH).transpose(1, 0, 2)
            ).reshape(P, NKB * SEQ_SH)
        if oi not in w_dev:
            ws = w8[oi * OUT_SH:(oi + 1) * OUT_SH, :]
            w_dev[oi] = np.ascontiguousarray(
                ws.T.reshape(NKB, P, OUT_SH).transpose(1, 0, 2)
            ).reshape(P, NKB * OUT_SH).view(ml_dtypes.float8_e4m3)
        sc_core = scale.T[:, oi * NOB:(oi + 1) * NOB]        # [NKB, NOB]
        sc = np.ascontiguousarray(
            np.broadcast_to(sc_core.reshape(1, NKB * NOB), (P, NKB * NOB)))
        in_maps.append({"xt": x_dev[si], "wt": w_dev[oi], "sc": sc})
    return in_maps


def unshard_output(results):
    out = np.empty((1, SEQ, DOUT), dtype=np.float32)
    for c in range(N_CORES):
        si, oi = c % SEQ_SHARDS, c // SEQ_SHARDS
        o = results[c]["out"].reshape(P, NSB, OUT_SH).transpose(1, 0, 2)
        out[0, si * SEQ_SH:(si + 1) * SEQ_SH,
            oi * OUT_SH:(oi + 1) * OUT_SH] = o.reshape(SEQ_SH, OUT_SH)
    return out


_NC_CACHE = {}


def _run_spmd(nc, in_maps, tries=3):
    """The axon-tunneled device occasionally faults with
    NRT_EXEC_UNIT_UNRECOVERABLE, which poisons the whole PJRT client —
    reset jax backends before retrying."""
    import time as _time
    last = None
    for t in range(tries):
        try:
            return run_bass_kernel_spmd(nc, in_maps, core_ids=list(range(N_CORES)))
        except Exception as e:  # noqa: BLE001
            last = e
            _time.sleep(2.0)
            try:
                import jax as _jax
                _jax.clear_backends()
            except Exception:  # noqa: BLE001
                pass
    raise last


def kernel(x, weight, weight_scale_inv):
    if "nc" not in _NC_CACHE:
        _NC_CACHE["nc"] = build_nc()
    nc = _NC_CACHE["nc"]
    in_maps = shard_inputs(x, weight, weight_scale_inv)
    res = _run_spmd(nc, in_maps)
    return unshard_output(res.results)
